# revision 20
# baseline (speedup 1.0000x reference)
"""Single-head causal attention kernel for Trainium2 (Bass/Tile), SPMD over 8 cores.

Problem: inputs [B=8, S=2048, E=1024]; Wq/Wk/Wv [E, H=1024]; bq/bk/bv [H].
  q = x@Wq+bq; k = x@Wk+bk; v = x@Wv+bv
  out = softmax(causal(q k^T / sqrt(H))) v        -> [B, S, H]

Sharding: data-parallel over batch, 1 batch element per NeuronCore (8 cores).

Strategy (all matmuls bf16 -> fp32 PSUM; ~4e-3 error vs 2e-2 gate):
  - host: x is pre-transposed to xT and cast to bf16, laid out so every DMA
    reads 8-16KB contiguous per partition; weights cast to bf16. No on-device
    transposes, everything SBUF-resident, one continuous PE stream.
  - projections: K^T/Q^T = W^T @ xT per 512-col s-chunk (bias fused into the
    PSUM eviction); V = xT.T @ Wv, bias-free: since softmax rows sum to 1,
    o = attn@(X Wv) + bv, so bv is folded into the final output eviction.
  - attention per 256-wide q-chunk: scores^T[k,q] (causal tiles skipped; the
    diagonal-straddling tile computed at half width), exp(x/32) fused on
    ScalarE -> bf16 attnT, edge mask via gpsimd.affine_select.
  - Z and O share the stationary operand: per k-tile, LDW(attnT) feeds two
    N=512 O-matmuls plus one N=1 ones-column matmul (row sums).
  - final eviction: one scalar_tensor_tensor per h-half: out = op*(1/Z) + bv.
  - DMA: xt chunks + out on sync queue, wk (h-halves) + wq on scalar queue,
    biases + wv on gpsimd queue, so the V phase never waits behind wk/wq.
"""

from contextlib import ExitStack

import numpy as np
import ml_dtypes

import concourse.bass as bass
import concourse.bacc as bacc
import concourse.mybir as mybir
from concourse import tile
from concourse import bass_utils

P = 128
F32 = mybir.dt.float32
BF16 = mybir.dt.bfloat16

B, S, E, H = 8, 2048, 1024, 1024
QC = 256          # q-chunk width in attention phase
N_CORES = 8


def attention_kernel(tc, out, xt, wq, bq, wk, bk, wv, bvf):
    nc = tc.nc
    ST, ET, HT = S // P, E // P, H // P     # 128-tiles per dim
    NSC = S // 512                          # 512-wide s-chunks
    NQC = S // QC                           # q-chunks
    HH = H // 2                             # 512-wide h-halves
    inv_sqrt_h = 1.0 / float(np.sqrt(H))
    Exp = mybir.ActivationFunctionType.Exp
    Ident = mybir.ActivationFunctionType.Identity
    Alu = mybir.AluOpType

    root = ExitStack()
    with root:
        # ---- constants ----
        const = root.enter_context(tc.tile_pool(name="const", bufs=1))
        ones_col = const.tile([P, 1], BF16, name="ones_col")
        nc.gpsimd.memset(ones_col, 1.0)
        bq_sb = const.tile([P, HT], F32, name="bq_sb")
        nc.gpsimd.dma_start(bq_sb[:], bq.rearrange("(t p) -> p t", p=P))
        bk_sb = const.tile([P, HT], F32, name="bk_sb")
        nc.gpsimd.dma_start(bk_sb[:], bk.rearrange("(t p) -> p t", p=P))
        bvf_sb = const.tile([P, H], BF16, name="bvf_sb")

        # ---- resident arrays: K^T, Q^T [h,s], V [s,h] (bf16) ----
        res_pool = root.enter_context(tc.tile_pool(name="res", bufs=1))
        kT = [res_pool.tile([P, S], BF16, name=f"kT{t}") for t in range(HT)]
        qT = [res_pool.tile([P, S], BF16, name=f"qT{t}") for t in range(HT)]
        v_sb = [res_pool.tile([P, H], BF16, name=f"v{i}") for i in range(ST)]

        # ================= phase 1: projections ================================
        with ExitStack() as ph:
            x_pool = ph.enter_context(tc.tile_pool(name="xt_sb", bufs=1))
            xt_c = [x_pool.tile([P, ET, 512], BF16, name=f"xt{c}")
                    for c in range(NSC)]
            w_pool = ph.enter_context(tc.tile_pool(name="w", bufs=1))
            wk_sb = w_pool.tile([P, HT, ET, P], BF16, name="wk_sb")
            wq_sb = w_pool.tile([P, ET, H], BF16, name="wq_sb")
            wv_sb = w_pool.tile([P, ET, H], BF16, name="wv_sb")

            # chunk 0 split across both hardware queues so it lands ~9us in;
            # wk as 512KB t-slices spread over all three queues so delivery
            # keeps pace with the (c=0, t) groups (consumed in t_order below).
            # The gpsimd queue (software DGE) kicks off ~14us in, so it only
            # carries slices needed late, then wv/bvf (needed much later).
            EH = ET // 2
            nc.sync.dma_start(xt_c[0][:, 0:EH, :], xt[0][:, 0:EH, :])
            nc.scalar.dma_start(xt_c[0][:, EH:ET, :], xt[0][:, EH:ET, :])
            for t in range(0, 4):
                nc.scalar.dma_start(wk_sb[:, t, :, :], wk[t])
            nc.sync.dma_start(xt_c[1][:], xt[1])
            nc.sync.dma_start(wk_sb[:, 4, :, :], wk[4])
            for t in range(5, 8):
                nc.gpsimd.dma_start(wk_sb[:, t, :, :], wk[t])
            for c in range(2, NSC):
                nc.sync.dma_start(xt_c[c][:], xt[c])
            nc.scalar.dma_start(wq_sb[:], wq[:])
            # wv + bvf behind wk t5-7 on the gpsimd queue; needed much later
            nc.gpsimd.dma_start(wv_sb[:], wv[:])
            nc.gpsimd.dma_start(bvf_sb[:], bvf[:])

            kqps = ph.enter_context(tc.tile_pool(name="kqps", bufs=4,
                                                 space="PSUM"))
            # K^T then Q^T: per s-chunk, per h-tile, accumulate over e
            wk_at = lambda t, e: wk_sb[:, t, e, :]
            wq_at = lambda t, e: wq_sb[:, e, t * P:(t + 1) * P]
            # chunk 0 of K^T eats t-slices in DMA-arrival order (gpsimd and
            # scalar queues deliver interleaved in time)
            c0_order = (0, 1, 5, 2, 6, 4, 3, 7)
            for w_at, dstT, b_sb in ((wk_at, kT, bk_sb), (wq_at, qT, bq_sb)):
                for c in range(NSC):
                    t_order = c0_order if (w_at is wk_at and c == 0) \
                        else range(HT)
                    for t in t_order:
                        kp = kqps.tile([P, 512], F32, name="kp", space="PSUM")
                        for e in range(ET):
                            nc.tensor.matmul(
                                kp[:],
                                w_at(t, e),
                                xt_c[c][:, e, :],
                                start=(e == 0), stop=(e == ET - 1))
                        dst = dstT[t][:, c * 512:(c + 1) * 512]
                        if t % 2 == 0:
                            nc.scalar.activation(dst, kp[:], Ident,
                                                 bias=b_sb[:, t:t + 1])
                        else:
                            nc.vector.tensor_scalar_add(dst, kp[:],
                                                        b_sb[:, t:t + 1])

            # V[s,h] (bias-free): per s-tile, two h-halves
            vps = ph.enter_context(tc.tile_pool(name="vps", bufs=2,
                                                space="PSUM"))
            for i in range(ST):
                c, cc = divmod(i, 4)
                vp0 = vps.tile([P, HH], F32, name="vp0", space="PSUM")
                vp1 = vps.tile([P, HH], F32, name="vp1", space="PSUM")
                for e in range(ET):
                    xblk = xt_c[c][:, e, cc * P:(cc + 1) * P]
                    nc.tensor.matmul(vp0[:], xblk, wv_sb[:, e, 0:HH],
                                     start=(e == 0), stop=(e == ET - 1))
                    nc.tensor.matmul(vp1[:], xblk, wv_sb[:, e, HH:H],
                                     start=(e == 0), stop=(e == ET - 1))
                if i % 2 == 0:
                    nc.scalar.activation(v_sb[i][:, 0:HH], vp0[:], Ident)
                    nc.vector.tensor_copy(v_sb[i][:, HH:H], vp1[:])
                else:
                    nc.vector.tensor_copy(v_sb[i][:, 0:HH], vp0[:])
                    nc.scalar.activation(v_sb[i][:, HH:H], vp1[:], Ident)

        # ================= phase 2: attention ==================================
        with ExitStack() as ph2:
            attn_pool = ph2.enter_context(
                tc.tile_pool(name="attnT", bufs=(S // P) + 2))
            o_pool = ph2.enter_context(tc.tile_pool(name="o_stage", bufs=3))
            rz_pool = ph2.enter_context(tc.tile_pool(name="rz", bufs=4))
            spsum = ph2.enter_context(tc.tile_pool(name="spsum", bufs=2,
                                                   space="PSUM"))
            opsum = ph2.enter_context(tc.tile_pool(name="opsum", bufs=2,
                                                   space="PSUM"))
            zpsum = ph2.enter_context(tc.tile_pool(name="zpsum", bufs=2,
                                                   space="PSUM"))
            QSUB = QC // P                       # q-subtiles per chunk
            for j in range(NQC):
                nk = 2 * j + 2        # k-tiles incl. the half-width diagonal
                attnT = []
                for i in range(nk):
                    half = (i == 2 * j + 1)      # only q-cols 128:256 valid
                    lo = P if half else 0
                    sp = spsum.tile([P, QC], F32, name="sp", space="PSUM")
                    for t in range(HT):
                        nc.tensor.matmul(
                            sp[:, lo:QC],
                            kT[t][:, i * P:(i + 1) * P],
                            qT[t][:, j * QC + lo:(j + 1) * QC],
                            start=(t == 0), stop=(t == HT - 1))
                    at = attn_pool.tile([P, QC], BF16, name="at")
                    nc.scalar.activation(at[:, lo:QC], sp[:, lo:QC], Exp,
                                         scale=inv_sqrt_h)
                    if i == 2 * j:
                        # keep q >= k: (j*QC - i*P) + f - p >= 0
                        nc.gpsimd.affine_select(
                            out=at[:], in_=at[:],
                            compare_op=Alu.is_ge,
                            fill=0.0,
                            base=j * QC - i * P,
                            channel_multiplier=-1,
                            pattern=[[1, QC]])
                    elif half:
                        # on the valid half: keep f' >= p  (f' = f - 128)
                        nc.gpsimd.affine_select(
                            out=at[:, P:QC], in_=at[:, P:QC],
                            compare_op=Alu.is_ge,
                            fill=0.0,
                            base=0,
                            channel_multiplier=-1,
                            pattern=[[1, P]])
                    attnT.append(at)
                for qs in range(QSUB):
                    nk_eff = 2 * j + qs + 1      # causal limit for this row tile
                    op0 = opsum.tile([P, HH], F32, name="op0", space="PSUM")
                    op1 = opsum.tile([P, HH], F32, name="op1", space="PSUM")
                    zp = zpsum.tile([P, 1], F32, name="zp", space="PSUM")
                    for i in range(nk_eff):
                        lhs = attnT[i][:, qs * P:(qs + 1) * P]
                        st, sp_ = (i == 0), (i == nk_eff - 1)
                        nc.tensor.matmul(op0[:], lhs, v_sb[i][:, 0:HH],
                                         start=st, stop=sp_)
                        nc.tensor.matmul(op1[:], lhs, v_sb[i][:, HH:H],
                                         start=st, stop=sp_)
                        nc.tensor.matmul(zp[:], lhs, ones_col[:],
                                         start=st, stop=sp_)
                    rz = rz_pool.tile([P, 1], F32, name="rz")
                    nc.vector.reciprocal(rz[:], zp[:])
                    o_stage = o_pool.tile([P, H], F32, name="o_stage")
                    row = j * QC + qs * P
                    last = (j == NQC - 1 and qs == QSUB - 1)
                    # out = op * (1/Z) + bv; the very last tile goes in
                    # quarters so eviction and DMA pipeline at the tail
                    QW = H // 4 if last else HH
                    for q4 in range(H // QW):
                        lo, hi = q4 * QW, (q4 + 1) * QW
                        src = op0 if hi <= HH else op1
                        slo, shi = lo % HH, (hi - 1) % HH + 1
                        nc.vector.scalar_tensor_tensor(
                            o_stage[:, lo:hi], src[:, slo:shi], rz[:],
                            bvf_sb[:, lo:hi], op0=Alu.mult, op1=Alu.add)
                        eng = nc.sync if q4 % 2 == 0 else nc.scalar
                        eng.dma_start(out[row:row + P, lo:hi],
                                      o_stage[:, lo:hi])


def build_program(n_cores=N_CORES):
    nc = bacc.Bacc("TRN2", target_bir_lowering=False, debug=False,
                   num_devices=n_cores)
    NSC = S // 512
    ET = E // P
    xt = nc.dram_tensor("xt", [NSC, P, ET, 512], BF16,
                        kind="ExternalInput").ap()
    wq = nc.dram_tensor("wq", [P, ET, H], BF16, kind="ExternalInput").ap()
    bq = nc.dram_tensor("bq", [H], F32, kind="ExternalInput").ap()
    wk = nc.dram_tensor("wk", [H // P, P, ET, P], BF16,
                        kind="ExternalInput").ap()
    bk = nc.dram_tensor("bk", [H], F32, kind="ExternalInput").ap()
    wv = nc.dram_tensor("wv", [P, ET, H], BF16, kind="ExternalInput").ap()
    bvf = nc.dram_tensor("bvf", [P, H], BF16, kind="ExternalInput").ap()
    out = nc.dram_tensor("out", [S, H], F32, kind="ExternalOutput").ap()
    with tile.TileContext(nc) as tc:
        attention_kernel(tc, out, xt, wq, bq, wk, bk, wv, bvf)
    nc.compile()
    return nc


def kernel(inputs, Wq, bq, Wk, bk, Wv, bv, _trace=False, _tmpdir=None):
    bf = ml_dtypes.bfloat16
    ET, NSC = E // P, S // 512
    inputs = np.asarray(inputs, dtype=np.float32)
    # [p, e, h]: per-partition 16KB-contiguous DMA lines
    wqh = np.ascontiguousarray(
        np.asarray(Wq, np.float32).astype(bf).reshape(ET, P, H)
        .transpose(1, 0, 2))
    wvh = np.ascontiguousarray(
        np.asarray(Wv, np.float32).astype(bf).reshape(ET, P, H)
        .transpose(1, 0, 2))
    # wk split into h-tile slices: [t, p, e, 128]
    wkh = np.ascontiguousarray(
        np.asarray(Wk, np.float32).astype(bf).reshape(ET, P, H // P, P)
        .transpose(2, 1, 0, 3))
    bq32 = np.ascontiguousarray(bq, dtype=np.float32)
    bk32 = np.ascontiguousarray(bk, dtype=np.float32)
    bvf16 = np.ascontiguousarray(
        np.broadcast_to(np.asarray(bv, np.float32).astype(bf), (P, H)))
    nc = build_program()
    in_maps = []
    for c in range(N_CORES):
        # xT chunk-major: [c, p, e, s] -> per-chunk contiguous 1MB DMA
        xtc = np.ascontiguousarray(
            inputs[c].T.astype(bf).reshape(ET, P, NSC, 512)
            .transpose(2, 1, 0, 3))
        in_maps.append({
            "xt": xtc,
            "wq": wqh, "bq": bq32,
            "wk": wkh, "bk": bk32,
            "wv": wvh, "bvf": bvf16,
        })
    res = bass_utils.run_bass_kernel_spmd(
        nc, in_maps, core_ids=list(range(N_CORES)),
        trace=_trace, tmpdir=_tmpdir)
    out = np.stack([res.results[c]["out"] for c in range(N_CORES)], axis=0)
    if _trace:
        kernel.last_results = res
    return out


# revision 23
# speedup vs baseline: 1.0182x; 1.0182x over previous
"""Single-head causal attention kernel for Trainium2 (Bass/Tile), SPMD over 8 cores.

Problem: inputs [B=8, S=2048, E=1024]; Wq/Wk/Wv [E, H=1024]; bq/bk/bv [H].
  q = x@Wq+bq; k = x@Wk+bk; v = x@Wv+bv
  out = softmax(causal(q k^T / sqrt(H))) v        -> [B, S, H]

Sharding: data-parallel over batch, 1 batch element per NeuronCore (8 cores).

Strategy (all matmuls bf16 -> fp32 PSUM; ~4e-3 error vs 2e-2 gate):
  - host: x is pre-transposed to xT and cast to bf16, laid out so every DMA
    reads 8-16KB contiguous per partition; weights cast to bf16. No on-device
    transposes, everything SBUF-resident, one continuous PE stream.
  - projections: K^T/Q^T = W^T @ xT per 512-col s-chunk (bias fused into the
    PSUM eviction); V = xT.T @ Wv, bias-free: since softmax rows sum to 1,
    o = attn@(X Wv) + bv, so bv is folded into the final output eviction.
  - attention per 256-wide q-chunk: scores^T[k,q] (causal tiles skipped; the
    diagonal-straddling tile computed at half width), exp(x/32) fused on
    ScalarE -> bf16 attnT, edge mask via gpsimd.affine_select.
  - Z and O share the stationary operand: per k-tile, LDW(attnT) feeds two
    N=512 O-matmuls plus one N=1 ones-column matmul (row sums).
  - final eviction: one scalar_tensor_tensor per h-half: out = op*(1/Z) + bv.
  - DMA: xt chunks + out on sync queue, wk (h-halves) + wq on scalar queue,
    biases + wv on gpsimd queue, so the V phase never waits behind wk/wq.
"""

from contextlib import ExitStack

import numpy as np
import ml_dtypes

import concourse.bass as bass
import concourse.bacc as bacc
import concourse.mybir as mybir
from concourse import tile
from concourse import bass_utils

P = 128
F32 = mybir.dt.float32
BF16 = mybir.dt.bfloat16

B, S, E, H = 8, 2048, 1024, 1024
QC = 256          # q-chunk width in attention phase
N_CORES = 8


def attention_kernel(tc, out, xt, wq, bq, wk, bk, wv, bvf):
    nc = tc.nc
    ST, ET, HT = S // P, E // P, H // P     # 128-tiles per dim
    NSC = S // 512                          # 512-wide s-chunks
    NQC = S // QC                           # q-chunks
    HH = H // 2                             # 512-wide h-halves
    inv_sqrt_h = 1.0 / float(np.sqrt(H))
    Exp = mybir.ActivationFunctionType.Exp
    Ident = mybir.ActivationFunctionType.Identity
    Alu = mybir.AluOpType

    root = ExitStack()
    with root:
        # ---- constants ----
        const = root.enter_context(tc.tile_pool(name="const", bufs=1))
        ones_col = const.tile([P, 1], BF16, name="ones_col")
        nc.gpsimd.memset(ones_col, 1.0)
        bq_sb = const.tile([P, HT], F32, name="bq_sb")
        nc.gpsimd.dma_start(bq_sb[:], bq.rearrange("(t p) -> p t", p=P))
        bk_sb = const.tile([P, HT], F32, name="bk_sb")
        nc.gpsimd.dma_start(bk_sb[:], bk.rearrange("(t p) -> p t", p=P))
        bvf_sb = const.tile([P, H], BF16, name="bvf_sb")
        nc.gpsimd.dma_start(bvf_sb[:], bvf[:])

        # ---- resident arrays: K^T, Q^T [h,s], V [s,h] (bf16) ----
        res_pool = root.enter_context(tc.tile_pool(name="res", bufs=1))
        kT = [res_pool.tile([P, S], BF16, name=f"kT{t}") for t in range(HT)]
        qT = [res_pool.tile([P, S], BF16, name=f"qT{t}") for t in range(HT)]
        v_sb = [res_pool.tile([P, H], BF16, name=f"v{i}") for i in range(ST)]

        # ================= phase 1: projections ================================
        with ExitStack() as ph:
            x_pool = ph.enter_context(tc.tile_pool(name="xt_sb", bufs=1))
            xt_c = [x_pool.tile([P, ET, 512], BF16, name=f"xt{c}")
                    for c in range(NSC)]
            w_pool = ph.enter_context(tc.tile_pool(name="w", bufs=1))
            wk_sb = w_pool.tile([P, HT, ET, P], BF16, name="wk_sb")
            wq_sb = w_pool.tile([P, ET, H], BF16, name="wq_sb")
            wv_sb = w_pool.tile([P, ET, H], BF16, name="wv_sb")

            # chunk 0 split across both hardware queues so it lands ~9us in;
            # wk as 512KB t-slices spread over all three queues so delivery
            # keeps pace with the (c=0, t) groups (consumed in t_order below).
            # The gpsimd queue (software DGE) kicks off ~14us in, so it only
            # carries slices needed late, then wv/bvf (needed much later).
            EH = ET // 2
            nc.sync.dma_start(xt_c[0][:, 0:EH, :], xt[0][:, 0:EH, :])
            nc.scalar.dma_start(xt_c[0][:, EH:ET, :], xt[0][:, EH:ET, :])
            for c in range(1, NSC):
                nc.sync.dma_start(xt_c[c][:], xt[c])
            for t in range(0, HT // 2):
                nc.scalar.dma_start(wk_sb[:, t, :, :], wk[t])
            for t in range(HT // 2, HT):
                nc.gpsimd.dma_start(wk_sb[:, t, :, :], wk[t])
            nc.scalar.dma_start(wq_sb[:], wq[:])
            # wv behind wk t4-7 on the gpsimd queue; needed only for phase V
            nc.gpsimd.dma_start(wv_sb[:], wv[:])

            kqps = ph.enter_context(tc.tile_pool(name="kqps", bufs=4,
                                                 space="PSUM"))
            # K^T then Q^T: per s-chunk, per h-tile, accumulate over e
            wk_at = lambda t, e: wk_sb[:, t, e, :]
            wq_at = lambda t, e: wq_sb[:, e, t * P:(t + 1) * P]
            # chunk 0 of K^T eats t-slices in DMA-arrival order (gpsimd and
            # scalar queues deliver interleaved in time)
            c0_order = (4, 0, 5, 1, 6, 2, 7, 3)
            for w_at, dstT, b_sb in ((wk_at, kT, bk_sb), (wq_at, qT, bq_sb)):
                for c in range(NSC):
                    t_order = c0_order if (w_at is wk_at and c == 0) \
                        else range(HT)
                    for t in t_order:
                        kp = kqps.tile([P, 512], F32, name="kp", space="PSUM")
                        for e in range(ET):
                            nc.tensor.matmul(
                                kp[:],
                                w_at(t, e),
                                xt_c[c][:, e, :],
                                start=(e == 0), stop=(e == ET - 1))
                        dst = dstT[t][:, c * 512:(c + 1) * 512]
                        if t % 2 == 0:
                            nc.scalar.activation(dst, kp[:], Ident,
                                                 bias=b_sb[:, t:t + 1])
                        else:
                            nc.vector.tensor_scalar_add(dst, kp[:],
                                                        b_sb[:, t:t + 1])

            # V[s,h] (bias-free): per s-tile, two h-halves
            vps = ph.enter_context(tc.tile_pool(name="vps", bufs=2,
                                                space="PSUM"))
            for i in range(ST):
                c, cc = divmod(i, 4)
                vp0 = vps.tile([P, HH], F32, name="vp0", space="PSUM")
                vp1 = vps.tile([P, HH], F32, name="vp1", space="PSUM")
                for e in range(ET):
                    xblk = xt_c[c][:, e, cc * P:(cc + 1) * P]
                    nc.tensor.matmul(vp0[:], xblk, wv_sb[:, e, 0:HH],
                                     start=(e == 0), stop=(e == ET - 1))
                    nc.tensor.matmul(vp1[:], xblk, wv_sb[:, e, HH:H],
                                     start=(e == 0), stop=(e == ET - 1))
                if i % 2 == 0:
                    nc.scalar.activation(v_sb[i][:, 0:HH], vp0[:], Ident)
                    nc.vector.tensor_copy(v_sb[i][:, HH:H], vp1[:])
                else:
                    nc.vector.tensor_copy(v_sb[i][:, 0:HH], vp0[:])
                    nc.scalar.activation(v_sb[i][:, HH:H], vp1[:], Ident)

        # ================= phase 2: attention ==================================
        with ExitStack() as ph2:
            attn_pool = ph2.enter_context(
                tc.tile_pool(name="attnT", bufs=(S // P) + 2))
            o_pool = ph2.enter_context(tc.tile_pool(name="o_stage", bufs=3))
            rz_pool = ph2.enter_context(tc.tile_pool(name="rz", bufs=4))
            spsum = ph2.enter_context(tc.tile_pool(name="spsum", bufs=2,
                                                   space="PSUM"))
            opsum = ph2.enter_context(tc.tile_pool(name="opsum", bufs=2,
                                                   space="PSUM"))
            zpsum = ph2.enter_context(tc.tile_pool(name="zpsum", bufs=2,
                                                   space="PSUM"))
            QSUB = QC // P                       # q-subtiles per chunk
            for j in range(NQC):
                nk = 2 * j + 2        # k-tiles incl. the half-width diagonal
                attnT = []
                for i in range(nk):
                    half = (i == 2 * j + 1)      # only q-cols 128:256 valid
                    lo = P if half else 0
                    sp = spsum.tile([P, QC], F32, name="sp", space="PSUM")
                    for t in range(HT):
                        nc.tensor.matmul(
                            sp[:, lo:QC],
                            kT[t][:, i * P:(i + 1) * P],
                            qT[t][:, j * QC + lo:(j + 1) * QC],
                            start=(t == 0), stop=(t == HT - 1))
                    at = attn_pool.tile([P, QC], BF16, name="at")
                    nc.scalar.activation(at[:, lo:QC], sp[:, lo:QC], Exp,
                                         scale=inv_sqrt_h)
                    if i == 2 * j:
                        # keep q >= k: (j*QC - i*P) + f - p >= 0
                        nc.gpsimd.affine_select(
                            out=at[:], in_=at[:],
                            compare_op=Alu.is_ge,
                            fill=0.0,
                            base=j * QC - i * P,
                            channel_multiplier=-1,
                            pattern=[[1, QC]])
                    elif half:
                        # on the valid half: keep f' >= p  (f' = f - 128)
                        nc.gpsimd.affine_select(
                            out=at[:, P:QC], in_=at[:, P:QC],
                            compare_op=Alu.is_ge,
                            fill=0.0,
                            base=0,
                            channel_multiplier=-1,
                            pattern=[[1, P]])
                    attnT.append(at)
                for qs in range(QSUB):
                    nk_eff = 2 * j + qs + 1      # causal limit for this row tile
                    op0 = opsum.tile([P, HH], F32, name="op0", space="PSUM")
                    op1 = opsum.tile([P, HH], F32, name="op1", space="PSUM")
                    zp = zpsum.tile([P, 1], F32, name="zp", space="PSUM")
                    for i in range(nk_eff):
                        lhs = attnT[i][:, qs * P:(qs + 1) * P]
                        st, sp_ = (i == 0), (i == nk_eff - 1)
                        nc.tensor.matmul(op0[:], lhs, v_sb[i][:, 0:HH],
                                         start=st, stop=sp_)
                        nc.tensor.matmul(op1[:], lhs, v_sb[i][:, HH:H],
                                         start=st, stop=sp_)
                        nc.tensor.matmul(zp[:], lhs, ones_col[:],
                                         start=st, stop=sp_)
                    rz = rz_pool.tile([P, 1], F32, name="rz")
                    nc.vector.reciprocal(rz[:], zp[:])
                    o_stage = o_pool.tile([P, H], F32, name="o_stage")
                    row = j * QC + qs * P
                    last = (j == NQC - 1 and qs == QSUB - 1)
                    # out = op * (1/Z) + bv; the very last tile goes in
                    # quarters so eviction and DMA pipeline at the tail
                    QW = H // 4 if last else HH
                    for q4 in range(H // QW):
                        lo, hi = q4 * QW, (q4 + 1) * QW
                        src = op0 if hi <= HH else op1
                        slo, shi = lo % HH, (hi - 1) % HH + 1
                        nc.vector.scalar_tensor_tensor(
                            o_stage[:, lo:hi], src[:, slo:shi], rz[:],
                            bvf_sb[:, lo:hi], op0=Alu.mult, op1=Alu.add)
                        eng = nc.sync if q4 % 2 == 0 else nc.scalar
                        eng.dma_start(out[row:row + P, lo:hi],
                                      o_stage[:, lo:hi])


def build_program(n_cores=N_CORES):
    nc = bacc.Bacc("TRN2", target_bir_lowering=False, debug=False,
                   num_devices=n_cores)
    NSC = S // 512
    ET = E // P
    xt = nc.dram_tensor("xt", [NSC, P, ET, 512], BF16,
                        kind="ExternalInput").ap()
    wq = nc.dram_tensor("wq", [P, ET, H], BF16, kind="ExternalInput").ap()
    bq = nc.dram_tensor("bq", [H], F32, kind="ExternalInput").ap()
    wk = nc.dram_tensor("wk", [H // P, P, ET, P], BF16,
                        kind="ExternalInput").ap()
    bk = nc.dram_tensor("bk", [H], F32, kind="ExternalInput").ap()
    wv = nc.dram_tensor("wv", [P, ET, H], BF16, kind="ExternalInput").ap()
    bvf = nc.dram_tensor("bvf", [P, H], BF16, kind="ExternalInput").ap()
    out = nc.dram_tensor("out", [S, H], F32, kind="ExternalOutput").ap()
    with tile.TileContext(nc) as tc:
        attention_kernel(tc, out, xt, wq, bq, wk, bk, wv, bvf)
    nc.compile()
    return nc


def kernel(inputs, Wq, bq, Wk, bk, Wv, bv, _trace=False, _tmpdir=None):
    bf = ml_dtypes.bfloat16
    ET, NSC = E // P, S // 512
    inputs = np.asarray(inputs, dtype=np.float32)
    # [p, e, h]: per-partition 16KB-contiguous DMA lines
    wqh = np.ascontiguousarray(
        np.asarray(Wq, np.float32).astype(bf).reshape(ET, P, H)
        .transpose(1, 0, 2))
    wvh = np.ascontiguousarray(
        np.asarray(Wv, np.float32).astype(bf).reshape(ET, P, H)
        .transpose(1, 0, 2))
    # wk split into h-tile slices: [t, p, e, 128]
    wkh = np.ascontiguousarray(
        np.asarray(Wk, np.float32).astype(bf).reshape(ET, P, H // P, P)
        .transpose(2, 1, 0, 3))
    bq32 = np.ascontiguousarray(bq, dtype=np.float32)
    bk32 = np.ascontiguousarray(bk, dtype=np.float32)
    bvf16 = np.ascontiguousarray(
        np.broadcast_to(np.asarray(bv, np.float32).astype(bf), (P, H)))
    nc = build_program()
    in_maps = []
    for c in range(N_CORES):
        # xT chunk-major: [c, p, e, s] -> per-chunk contiguous 1MB DMA
        xtc = np.ascontiguousarray(
            inputs[c].T.astype(bf).reshape(ET, P, NSC, 512)
            .transpose(2, 1, 0, 3))
        in_maps.append({
            "xt": xtc,
            "wq": wqh, "bq": bq32,
            "wk": wkh, "bk": bk32,
            "wv": wvh, "bvf": bvf16,
        })
    res = bass_utils.run_bass_kernel_spmd(
        nc, in_maps, core_ids=list(range(N_CORES)),
        trace=_trace, tmpdir=_tmpdir)
    out = np.stack([res.results[c]["out"] for c in range(N_CORES)], axis=0)
    if _trace:
        kernel.last_results = res
    return out
